# revision 9
# baseline (speedup 1.0000x reference)
"""Trainium2 Bass kernel for GQA attention (B=4, L=2048, HID=896,
14 q-heads / 2 kv-heads, HD=64, RoPE + causal mask + o_proj).

Sharding: one NeuronCore per (batch, kv-head) pair -> exactly 8 shards of
7 q-heads each. o_proj is row-sharded; partials are summed with a pairwise
ReduceScatter in row chunks overlapped with compute.

v2 engine-balance rework (ScalarE exp was the bottleneck at ~207us busy):
- softmax exp split across engines: ScalarE keeps the a-sides, VectorE
  computes the b-sides of the first NSCHR head pairs with a one-instruction
  Schraudolph bit-trick (tensor_scalar -> int16 bits reinterpreted as bf16;
  the appended ones-column denominator sums the same approximated values, so
  the approximation acts as bounded logit noise, not output bias).
- normalization: one Ln over both sides' denominators, K=1 broadcast matmul,
  one Exp(-x) over the [64,1024] broadcast, two DVE muls (no more per-side
  [1,512] activations and no DVE CAST).
- RoPE: PSUM->SBUF casts moved to ScalarE (idle during projections) with the
  q/k bias folded into the activation bias port (bias matmuls dropped);
  cos/sin muls run full-L in bf16 on DVE (2x mode) instead of 5 fp32
  PSUM-source muls per 512-block.
- causal diag mask: one batched bf16 DVE mul over both sides (was 2 GpSimd
  muls on the exp->PV critical path).
- o_proj PSUM->SBUF copies alternate ScalarE/VectorE.
- ReduceScatter tail chunks shrunk (last chunk 128 rows).
"""
import os
import sys

sys.path.insert(0, "/opt/trn_rl_repo")

import numpy as np
import ml_dtypes

import concourse.bass as bass
import concourse.mybir as mybir
import concourse.tile as tile
from concourse.bass_utils import run_bass_kernel_spmd

BF16NP = ml_dtypes.bfloat16
F32 = mybir.dt.float32
BF16 = mybir.dt.bfloat16
I16 = mybir.dt.int16

B, L, HID = 4, 2048, 896
NH, NKV, HD = 14, 2, 64
HPC = NH // NKV  # heads per core = 7
NCORES = 8
KCH = HID // 128  # 7 contraction chunks
NIB = L // 512  # 4 i-blocks
NJC = L // 128  # 16 j-chunks
NSCHR = int(os.environ.get("NSCHR", "3"))  # pairs whose b-side exp -> DVE
SCHR_A = 23.08312051  # 0.125 (softmax scale) * 128 * log2(e)
SCHR_B = float(os.environ.get("SCHR_B", "16250.5"))
MASK_DVE = bool(int(os.environ.get("MASK_DVE", "1")))
# RS row chunks; small tail chunks so the last (unoverlapped) collective
# is cheap (~3.1us fixed + ~14ns/row measured)
CH_ROWS = [(0, 512), (512, 512), (1024, 512), (1536, 256), (1792, 128),
           (1920, 128)]


def _kc_of(mt):
    r = 128 * mt
    for k, (s, n) in enumerate(CH_ROWS):
        if s <= r < s + n:
            return k
    raise AssertionError


def _fix_drains(nc, maxw=1):
    """This walrus build allows a single sync-wait per instruction; hoist
    excess waits onto preceding single-wait Drain instructions on the same
    engine (engine streams are in-order, so semantics are preserved)."""
    n = 0
    for fn in nc.m.functions:
        for blk in fn.blocks:
            newlist = []
            for ins in blk.instructions:
                si = getattr(ins, "sync_info", None)
                ow = list(si.on_wait) if si is not None and si.on_wait else []
                if len(ow) > maxw:
                    rest = ow[:]
                    while len(rest) > maxw:
                        chunk, rest = rest[:maxw], rest[maxw:]
                        d = mybir.InstNoOp(
                            name=f"{ins.name}-ws{n}", ins=[], outs=[]
                        )
                        d.engine = ins.engine
                        d.sync_info = mybir.SyncInfo(on_wait=chunk, on_update=[])
                        newlist.append(d)
                        n += 1
                    si.on_wait = rest
                newlist.append(ins)
            blk.instructions = newlist
    return n


def build():
    nc = bass.Bass("TRN2", num_devices=NCORES, debug=False)

    xt_d = nc.dram_tensor("xt", [128, KCH, L], BF16, kind="ExternalInput")
    wq_d = nc.dram_tensor("wq", [128, KCH, 448], BF16, kind="ExternalInput")
    qb_d = nc.dram_tensor("qb", [128, 4], F32, kind="ExternalInput")
    wk_d = nc.dram_tensor("wk", [128, KCH, 128], BF16, kind="ExternalInput")
    kb_d = nc.dram_tensor("kb", [128, 1], F32, kind="ExternalInput")
    wv_d = nc.dram_tensor("wv", [128, KCH, 64], BF16, kind="ExternalInput")
    wvb_d = nc.dram_tensor("wvb", [1, 64], BF16, kind="ExternalInput")
    wo_d = nc.dram_tensor("wo", [128, 4, HID], BF16, kind="ExternalInput")
    cos_d = nc.dram_tensor("cos", [128, L], BF16, kind="ExternalInput")
    sin_d = nc.dram_tensor("sin", [128, L], BF16, kind="ExternalInput")
    mask_d = nc.dram_tensor("mask", [128, 2, 128], BF16, kind="ExternalInput")
    out_d = nc.dram_tensor("out", [L // 2, HID], BF16, kind="ExternalOutput")

    EXP = mybir.ActivationFunctionType.Exp
    LN = mybir.ActivationFunctionType.Ln
    IDT = mybir.ActivationFunctionType.Identity

    with tile.TileContext(nc) as tc:
        with (
            tc.tile_pool(name="const", bufs=1) as cpool,
            tc.tile_pool(name="qt", bufs=4) as qtpool,
            tc.tile_pool(name="per", bufs=1) as perpool,
            tc.tile_pool(name="ot", bufs=4) as otpool,
            tc.tile_pool(name="rraw", bufs=2) as rrawp,
            tc.tile_pool(name="rt1", bufs=2) as rt1p,
            tc.tile_pool(name="rt2", bufs=2) as rt2p,
            tc.tile_pool(name="ptp", bufs=8) as ptp,
            tc.tile_pool(name="nrm", bufs=4) as nrm,
            tc.tile_pool(name="osb", bufs=4) as osbp,
            tc.tile_pool(name="ps_sp", bufs=2, space="PSUM") as ps_sp,
            tc.tile_pool(name="ps_o", bufs=2, space="PSUM") as ps_o,
            tc.tile_pool(name="dram", bufs=1, space="DRAM") as drpool,
        ):
            # ---- constants / inputs to SBUF (first consumers first) ----
            wk = cpool.tile([128, KCH, 128], BF16, tag="wk")
            nc.sync.dma_start(wk[:], wk_d.ap())
            kb = cpool.tile([128, 1], F32, tag="kb")
            nc.sync.dma_start(kb[:], kb_d.ap())
            xt = cpool.tile([128, KCH, L], BF16, tag="xt")
            # band-major so K-proj m=0 can start after ~1/4 of the x DMA
            for m in range(4):
                for k in range(KCH):
                    ms = bass.ts(m, 512)
                    nc.sync.dma_start(xt[:, k, ms], xt_d.ap()[:, k, ms])
            cosb = cpool.tile([128, L], BF16, tag="cosb")
            nc.sync.dma_start(cosb[:], cos_d.ap())
            sinb = cpool.tile([128, L], BF16, tag="sinb")
            nc.sync.dma_start(sinb[:], sin_d.ap())
            wv = cpool.tile([128, KCH, 64], BF16, tag="wv")
            nc.sync.dma_start(wv[:], wv_d.ap())
            wvb = cpool.tile([1, 64], BF16, tag="wvb")
            nc.sync.dma_start(wvb[:], wvb_d.ap())
            wq = cpool.tile([128, KCH, 448], BF16, tag="wq")
            nc.sync.dma_start(wq[:], wq_d.ap())
            qb = cpool.tile([128, 4], F32, tag="qb")
            nc.sync.dma_start(qb[:], qb_d.ap())
            msk = cpool.tile([128, 2, 128], BF16, tag="msk")
            nc.sync.dma_start(msk[:], mask_d.ap())
            wo = cpool.tile([128, 4, HID], BF16, tag="wo")
            nc.sync.dma_start(wo[:], wo_d.ap())
            ones_row = cpool.tile([1, L], BF16, tag="ones_row")
            nc.vector.memset(ones_row[:], 1.0)
            ones64 = cpool.tile([1, 64], BF16, tag="ones64")
            nc.vector.memset(ones64[:], 1.0)

            # PE warmup: ~7.5us of dummy matmuls while the input DMAs land,
            # so the HAM clock-gate reaches 8/8 before the projections start.
            warm = ps_o.tile([128, 1024], F32, tag="o", name="warm")
            for _ in range(70):
                nc.tensor.matmul(warm[:, 0:128], ones_row[0:1, 0:128],
                                 ones_row[0:1, 0:128], start=True, stop=True)

            partials = [
                drpool.tile([n, HID], BF16, tag=f"partial{k}",
                            name=f"partial{k}")
                for k, (_, n) in enumerate(CH_ROWS)
            ]
            shards = [
                drpool.tile([n // 2, HID], BF16, tag=f"shard{k}",
                            name=f"shard{k}")
                for k, (_, n) in enumerate(CH_ROWS)
            ]

            def rope(dst, raw, P, hs):
                """dst[0:P, hs] = (raw*cos + rotate_half(raw)*sin)[:, hs] in
                bf16 (DVE 2x mode), called per half-L so qt bands are ready
                early; the rotate_half partition swap is cross-offset muls
                against the row-swapped sign-folded sin."""
                t1 = rt1p.tile([128, L // 2], BF16, tag="t1")
                nc.vector.tensor_mul(t1[0:P, :], raw[0:P, hs], cosb[0:P, hs])
                t2 = rt2p.tile([128, L // 2], BF16, tag="t2")
                for b in range(P // 32):
                    s = 32 * (b ^ 1)
                    nc.vector.tensor_mul(
                        t2[32 * b : 32 * b + 32, :],
                        raw[s : s + 32, hs],
                        sinb[s : s + 32, hs],
                    )
                nc.vector.tensor_add(dst[0:P, hs], t1[0:P, :], t2[0:P, :])

            # ---- K^T projection (duplicated across partitions) + RoPE ----
            kt = perpool.tile([128, L], BF16, tag="kt")
            kraw = rrawp.tile([128, L], BF16, tag="raw", name="kraw")
            for m in range(4):
                ms = bass.ts(m, 512)
                kp = ps_sp.tile([128, 1024], F32, tag="sp", name="kp")
                for k in range(KCH):
                    nc.tensor.matmul(kp[:, 0:512], wk[:, k, :], xt[:, k, ms],
                                     start=(k == 0), stop=(k == KCH - 1))
                # PSUM->SBUF bf16 cast with the k bias folded in
                nc.scalar.activation(kraw[:, ms], kp[:, 0:512], IDT,
                                     bias=kb[:, 0:1])
                if m % 2 == 1:
                    rope(kt, kraw, 128, bass.ts(m // 2, 1024))

            # ---- Q^T projections + RoPE (head pairs on 128 partitions),
            # ---- pair 0 first so band-0 attention starts before V lands ----
            qts = []

            def qproj(p):
                P = 128 if p < 3 else 64
                ns = bass.ds(128 * p, P)
                qt = qtpool.tile([128, L], BF16, tag="qt", name=f"qt{p}")
                qts.append(qt)
                qraw = rrawp.tile([128, L], BF16, tag="raw", name=f"qraw{p}")
                for m in range(4):
                    ms = bass.ts(m, 512)
                    qp = ps_sp.tile([128, 1024], F32, tag="sp", name="qp")
                    for k in range(KCH):
                        nc.tensor.matmul(qp[0:P, 0:512], wq[:, k, ns],
                                         xt[:, k, ms],
                                         start=(k == 0), stop=(k == KCH - 1))
                    nc.scalar.activation(qraw[0:P, ms], qp[0:P, 0:512], IDT,
                                         bias=qb[0:P, p : p + 1])
                    if m % 2 == 1:
                        rope(qt, qraw, P, bass.ts(m // 2, 1024))

            qproj(0)

            # ---- V projection (natural layout + ones column) ----
            vt = perpool.tile([128, NJC, 65], BF16, tag="vt")
            nc.vector.memset(vt[:, :, 64:65], 1.0)
            for mt in range(NJC):
                vp = ps_o.tile([128, 1024], F32, tag="o", name="vp")
                for k in range(KCH):
                    nc.tensor.matmul(vp[:, 0:64], xt[:, k, bass.ts(mt, 128)],
                                     wv[:, k, :], start=(k == 0), stop=False)
                nc.tensor.matmul(vp[:, 0:64], ones_row[0:1, bass.ts(mt, 128)],
                                 wvb[0:1, :], start=False, stop=True)
                nc.scalar.copy(vt[:, mt, 0:64], vp[:, 0:64])

            for p in range(1, 4):
                qproj(p)

            # ---- attention, head pairs packed on PE rows 0:64 / 64:128 ----
            # ib-major so each 512-row band of O^T completes early and its
            # o_proj + ReduceScatter chunk overlaps the next band's attention
            otp = [
                otpool.tile([128, L], BF16, tag="ot", name=f"otp{i}")
                for i in range(4)
            ]
            # Deferred emission: each pair's normalization and each band's
            # o_proj chunks are queued as closures and emitted one-per-jc
            # inside LATER pairs' attention streams, so the PE FIFO always
            # has S/PV work queued ahead of instructions that wait on the
            # ScalarE Ln/Exp chain (measured ~2-3us PE stall per pair-band
            # otherwise, plus ~10us ScalarE bubbles at band boundaries).
            deferred = []

            def flush_one():
                if deferred:
                    deferred.pop(0)()

            def norm_ln_closure(oab, has_b):
                # stage 1: just the Ln (its input is ready when flushed, so
                # it never blocks; the PE-side broadcast comes >=1 jc later)
                lnd = nrm.tile([1, 1024], BF16, tag="lnd")

                def emit():
                    W = 1024 if has_b else 512
                    nc.scalar.activation(lnd[0:1, 0:W], oab[64:65, 0:W], LN)
                return lnd, emit

            def norm_rb_closure(p, ib, oab, lnd, has_b):
                def emit():
                    W = 1024 if has_b else 512
                    rb = ps_sp.tile([128, 1024], F32, tag="sp", name="rb")
                    nc.tensor.matmul(rb[0:64, 0:512], ones64[0:1, :],
                                     lnd[0:1, 0:512], start=True, stop=True)
                    if has_b:
                        nc.tensor.matmul(rb[0:64, 512:1024], ones64[0:1, :],
                                         lnd[0:1, 512:1024],
                                         start=True, stop=True)
                    rbs = nrm.tile([64, 1024], BF16, tag="rbs")
                    nc.scalar.activation(rbs[:, 0:W], rb[0:64, 0:W], EXP,
                                         scale=-1.0)
                    ibb = bass.ts(ib, 512)
                    nc.vector.tensor_mul(otp[p][0:64, ibb],
                                         oab[0:64, 0:512], rbs[:, 0:512])
                    if has_b:
                        nc.vector.tensor_mul(otp[p][64:128, ibb],
                                             oab[0:64, 512:1024],
                                             rbs[:, 512:1024])
                return emit

            def oproj_closure(mt, ch):
                def emit():
                    msl = bass.ts(mt, 128)
                    kc = _kc_of(mt)
                    csl = bass.ts(ch, 448)
                    op_ = ps_sp.tile([128, 1024], F32, tag="sp", name="opj")
                    for p in range(4):
                        P = 128 if p < 3 else 64
                        nc.tensor.matmul(
                            op_[:, 0:448], otp[p][0:P, msl],
                            wo[0:P, p, csl],
                            start=(p == 0), stop=(p == 3),
                        )
                    osb = osbp.tile([128, 448], BF16, tag="osb")
                    if ch == 0:
                        nc.scalar.copy(osb[:, :], op_[:, 0:448])
                    else:
                        nc.vector.tensor_copy(osb[:, :], op_[:, 0:448])
                    row0 = 128 * mt - CH_ROWS[kc][0]
                    nc.sync.dma_start(
                        partials[kc][bass.ds(row0, 128), csl],
                        osb[:, :],
                    )
                    if ch == 1 and (
                        128 * mt + 128 == CH_ROWS[kc][0] + CH_ROWS[kc][1]
                    ):
                        nc.gpsimd.collective_compute(
                            "ReduceScatter",
                            mybir.AluOpType.add,
                            ins=[partials[kc].opt()],
                            outs=[shards[kc].opt()],
                            replica_groups=[[0, 1], [2, 3], [4, 5], [6, 7]],
                        )
                        nc.sync.dma_start(
                            out_d.ap()[
                                bass.ds(CH_ROWS[kc][0] // 2,
                                        CH_ROWS[kc][1] // 2), :
                            ],
                            shards[kc][:, :],
                        )
                return emit

            for ib in range(NIB):
                i0 = 512 * ib
                for p in range(4):
                    qt = qts[p]
                    has_b = p < 3
                    schr_b = has_b and p < NSCHR
                    # sides a/b accumulate in one 2-bank PSUM tile:
                    # cols 0:512 side a, 512:1024 side b; row 64 = denominators
                    oab = ps_o.tile([128, 1024], F32, tag="o", name="oab")
                    njc = 4 * ib + 4
                    for jc in range(njc):
                        t = jc - 4 * ib  # >=0 on the diagonal blocks
                        c0 = 128 * t if t >= 0 else 0
                        cw = 512 - c0
                        cs = bass.ds(c0, cw)
                        isl = bass.ds(i0 + c0, cw)
                        jsl = bass.ts(jc, 128)
                        sp = ps_sp.tile([128, 1024], F32, tag="sp")
                        nc.tensor.matmul(sp[:, 0:512][:, cs], kt[0:64, jsl],
                                         qt[0:64, isl], start=True, stop=True)
                        if has_b:
                            nc.tensor.matmul(sp[:, 512:1024][:, cs],
                                             kt[64:128, jsl], qt[64:128, isl],
                                             start=True, stop=True)
                        pt = ptp.tile([128, 1024], BF16, tag="pt")
                        # --- exp: side a on ScalarE, side b on ScalarE or
                        # --- VectorE (Schraudolph bit-trick) ---
                        if schr_b:
                            nc.scalar.activation(pt[:, 0:512][:, cs],
                                                 sp[:, 0:512][:, cs], EXP,
                                                 scale=0.125)
                            nc.vector.tensor_scalar(
                                pt[:, 512:1024][:, cs].bitcast(I16),
                                sp[:, 512:1024][:, cs],
                                SCHR_A, SCHR_B,
                                mybir.AluOpType.mult, mybir.AluOpType.add,
                            )
                        elif has_b and t < 0:
                            nc.scalar.activation(pt[:, :], sp[:, :], EXP,
                                                 scale=0.125)
                        elif has_b:
                            sp3 = sp.rearrange("p (s c) -> p s c", s=2)
                            pt3 = pt.rearrange("p (s c) -> p s c", s=2)
                            nc.scalar.activation(pt3[:, :, c0:512],
                                                 sp3[:, :, c0:512], EXP,
                                                 scale=0.125)
                        else:
                            nc.scalar.activation(pt[:, cs], sp[:, 0:512][:, cs],
                                                 EXP, scale=0.125)
                        if t >= 0:
                            # zero the invalid upper triangle of the diagonal
                            # sub-tile (0/1 mask)
                            pt3 = pt.rearrange("p (s c) -> p s c", s=2)
                            dcs = bass.ds(c0, 128)
                            if MASK_DVE:
                                if has_b:
                                    nc.vector.tensor_mul(pt3[:, :, dcs],
                                                         pt3[:, :, dcs],
                                                         msk[:, :, :])
                                else:
                                    nc.vector.tensor_mul(pt[:, dcs],
                                                         pt[:, dcs],
                                                         msk[:, 0, :])
                            else:
                                nc.gpsimd.tensor_mul(pt[:, dcs], pt[:, dcs],
                                                     msk[:, 0, :])
                                if has_b:
                                    dcs2 = bass.ds(512 + c0, 128)
                                    nc.gpsimd.tensor_mul(pt[:, dcs2],
                                                         pt[:, dcs2],
                                                         msk[:, 0, :])
                        nc.tensor.matmul(oab[0:65, 0:512][:, cs], vt[:, jc, :],
                                         pt[:, cs],
                                         start=(jc == 0), stop=(jc == njc - 1))
                        if has_b:
                            nc.tensor.matmul(oab[0:65, 512:1024][:, cs],
                                             vt[:, jc, :],
                                             pt[:, 512:1024][:, cs],
                                             start=(jc == 0), stop=(jc == njc - 1))
                        if jc >= 1:
                            flush_one()
                    lnd, ln_emit = norm_ln_closure(oab, has_b)
                    deferred.append(ln_emit)
                    deferred.append(norm_rb_closure(p, ib, oab, lnd, has_b))
                for mt in range(4 * ib, 4 * ib + 4):
                    for ch in range(2):
                        deferred.append(oproj_closure(mt, ch))
            while deferred:
                flush_one()

    _fix_drains(nc)
    return nc


def _kpack(wT):
    """[896, N] f32 -> [128, 7, N] bf16 contiguous (k-chunked)."""
    n = wT.shape[1]
    return np.ascontiguousarray(
        wT.reshape(KCH, 128, n).transpose(1, 0, 2).astype(BF16NP)
    )


def _wopack(wo_s):
    """wo shard [896, 448] -> [128, 4, 896] bf16: per head-pair p,
    partitions hold that pair's 128 rows of woT (= wo_s.T)."""
    woT = wo_s.T  # [448, 896]
    out = np.zeros((128, 4, HID), dtype=BF16NP)
    for p in range(4):
        rows = woT[128 * p : min(128 * p + 128, 448)]
        out[: rows.shape[0], p, :] = rows.astype(BF16NP)
    return out


_CACHE = {}


def kernel(**inputs):
    x = np.asarray(inputs["x"], dtype=np.float32)
    cos = np.asarray(inputs["cos"], dtype=np.float32)
    sin = np.asarray(inputs["sin"], dtype=np.float32)
    mask = np.asarray(inputs["mask"], dtype=np.float32)
    wq = np.asarray(inputs["wq"], dtype=np.float32)
    bq = np.asarray(inputs["bq"], dtype=np.float32)
    wk = np.asarray(inputs["wk"], dtype=np.float32)
    bk = np.asarray(inputs["bk"], dtype=np.float32)
    wv = np.asarray(inputs["wv"], dtype=np.float32)
    bv = np.asarray(inputs["bv"], dtype=np.float32)
    wo = np.asarray(inputs["wo"], dtype=np.float32)

    cosT = np.ascontiguousarray(
        np.tile(cos[0, 0].T, (2, 1)).astype(BF16NP))  # [128, L]
    sinT = sin[0, 0].T  # [64, L]
    # sign-folded sin for the in-place rotate_half: out[32b:32b+32] reads
    # q[32(b^1):...] times these rows; rows 0:32 carry the minus sign
    # rows pre-swapped in 32-blocks: the rope mul reads raw[s:s+32] and
    # sin rows s:s+32 and writes t2 rows 32b (s = 32*(b^1)); rows carrying
    # the minus sign are the (pre-swap) first half of each head
    sinm = np.ascontiguousarray(
        np.tile(np.concatenate([sinT[32:64], -sinT[0:32]], axis=0),
                (2, 1)).astype(BF16NP)
    )
    mask_diag = (mask[0, 0, :128, :128].T == 0.0).astype(BF16NP)
    mask2 = np.ascontiguousarray(
        np.stack([mask_diag, mask_diag], axis=1))  # [128, 2, 128]

    in_maps = []
    for core in range(NCORES):
        b, g = divmod(core, NKV)
        wq_s = wq[448 * g : 448 * (g + 1)]
        bq_s = bq[448 * g : 448 * (g + 1)]
        wk_s = wk[64 * g : 64 * (g + 1)]
        bk_s = bk[64 * g : 64 * (g + 1)]
        wv_s = wv[64 * g : 64 * (g + 1)]
        bv_s = bv[64 * g : 64 * (g + 1)]
        wo_s = wo[:, 448 * g : 448 * (g + 1)]  # [896, 448]
        wk_dup = np.concatenate([wk_s, wk_s], axis=0)  # [128, 896]
        bk_dup = np.concatenate([bk_s, bk_s], axis=0)
        qb2 = np.zeros((128, 4), dtype=np.float32)
        for p in range(4):
            rows = bq_s[128 * p : min(128 * p + 128, 448)]
            qb2[: rows.shape[0], p] = rows
        in_maps.append({
            "xt": _kpack(x[b].T),
            "wq": _kpack(wq_s.T),
            "qb": qb2,
            "wk": _kpack(wk_dup.T),
            "kb": np.ascontiguousarray(bk_dup[:, None]),
            "wv": _kpack(wv_s.T),
            "wvb": bv_s.astype(BF16NP)[None, :],
            "wo": _wopack(wo_s),
            "cos": cosT,
            "sin": sinm,
            "mask": mask2,
        })

    if "nc" not in _CACHE:
        _CACHE["nc"] = build()
    trace = bool(os.environ.get("KERNEL_TRACE"))
    res = run_bass_kernel_spmd(
        _CACHE["nc"], in_maps, core_ids=list(range(NCORES)), trace=trace
    )
    global LAST_EXEC_NS
    LAST_EXEC_NS = res.exec_time_ns
    out = np.empty((B, L, HID), dtype=np.float32)
    for b in range(B):
        lo = res.results[2 * b]["out"].astype(np.float32)
        hi = res.results[2 * b + 1]["out"].astype(np.float32)
        for start, n in CH_ROWS:
            h = n // 2
            s2 = start // 2
            out[b, start : start + h] = lo[s2 : s2 + h]
            out[b, start + h : start + n] = hi[s2 : s2 + h]
    return out


LAST_EXEC_NS = None


# revision 11
# speedup vs baseline: 1.4163x; 1.4163x over previous
"""Trainium2 Bass kernel for GQA attention (B=4, L=2048, HID=896,
14 q-heads / 2 kv-heads, HD=64, RoPE + causal mask + o_proj).

Sharding: one NeuronCore per (batch, kv-head) pair -> exactly 8 shards of
7 q-heads each. o_proj is row-sharded; partials are summed with a pairwise
ReduceScatter in row chunks overlapped with compute.

v2 engine-balance rework (ScalarE exp was the bottleneck at ~207us busy):
- softmax exp split across engines: ScalarE keeps the a-sides, VectorE
  computes the b-sides of the first NSCHR head pairs with a one-instruction
  Schraudolph bit-trick (tensor_scalar -> int16 bits reinterpreted as bf16;
  the appended ones-column denominator sums the same approximated values, so
  the approximation acts as bounded logit noise, not output bias).
- normalization: one Ln over both sides' denominators, K=1 broadcast matmul,
  one Exp(-x) over the [64,1024] broadcast, two DVE muls (no more per-side
  [1,512] activations and no DVE CAST).
- RoPE: PSUM->SBUF casts moved to ScalarE (idle during projections) with the
  q/k bias folded into the activation bias port (bias matmuls dropped);
  cos/sin muls run full-L in bf16 on DVE (2x mode) instead of 5 fp32
  PSUM-source muls per 512-block.
- causal diag mask: one batched bf16 DVE mul over both sides (was 2 GpSimd
  muls on the exp->PV critical path).
- o_proj PSUM->SBUF copies alternate ScalarE/VectorE.
- ReduceScatter tail chunks shrunk (last chunk 128 rows).
"""
import os
import sys

sys.path.insert(0, "/opt/trn_rl_repo")

import numpy as np
import ml_dtypes

import concourse.bass as bass
import concourse.mybir as mybir
import concourse.tile as tile
from concourse.bass_utils import run_bass_kernel_spmd

BF16NP = ml_dtypes.bfloat16
F32 = mybir.dt.float32
BF16 = mybir.dt.bfloat16
I16 = mybir.dt.int16

B, L, HID = 4, 2048, 896
NH, NKV, HD = 14, 2, 64
HPC = NH // NKV  # heads per core = 7
NCORES = 8
KCH = HID // 128  # 7 contraction chunks
NIB = L // 512  # 4 i-blocks
NJC = L // 128  # 16 j-chunks
NSCHR = int(os.environ.get("NSCHR", "3"))  # pairs whose b-side exp -> DVE
SCHR_A = 23.08312051  # 0.125 (softmax scale) * 128 * log2(e)
SCHR_B = float(os.environ.get("SCHR_B", "16250.5"))
MASK_DVE = bool(int(os.environ.get("MASK_DVE", "1")))
# RS row chunks; small tail chunks so the last (unoverlapped) collective
# is cheap (~3.1us fixed + ~14ns/row measured)
CH_ROWS = [(0, 512), (512, 512), (1024, 512), (1536, 256), (1792, 128),
           (1920, 128)]


def _kc_of(mt):
    r = 128 * mt
    for k, (s, n) in enumerate(CH_ROWS):
        if s <= r < s + n:
            return k
    raise AssertionError


def _fix_drains(nc, maxw=1):
    """This walrus build allows a single sync-wait per instruction; hoist
    excess waits onto preceding single-wait Drain instructions on the same
    engine (engine streams are in-order, so semantics are preserved)."""
    n = 0
    for fn in nc.m.functions:
        for blk in fn.blocks:
            newlist = []
            for ins in blk.instructions:
                si = getattr(ins, "sync_info", None)
                ow = list(si.on_wait) if si is not None and si.on_wait else []
                if len(ow) > maxw:
                    rest = ow[:]
                    while len(rest) > maxw:
                        chunk, rest = rest[:maxw], rest[maxw:]
                        d = mybir.InstNoOp(
                            name=f"{ins.name}-ws{n}", ins=[], outs=[]
                        )
                        d.engine = ins.engine
                        d.sync_info = mybir.SyncInfo(on_wait=chunk, on_update=[])
                        newlist.append(d)
                        n += 1
                    si.on_wait = rest
                newlist.append(ins)
            blk.instructions = newlist
    return n


def build():
    nc = bass.Bass("TRN2", num_devices=NCORES, debug=False)

    xt_d = nc.dram_tensor("xt", [128, KCH, L], BF16, kind="ExternalInput")
    wq_d = nc.dram_tensor("wq", [128, KCH, 448], BF16, kind="ExternalInput")
    qb_d = nc.dram_tensor("qb", [128, 4], F32, kind="ExternalInput")
    wk_d = nc.dram_tensor("wk", [128, KCH, 128], BF16, kind="ExternalInput")
    kb_d = nc.dram_tensor("kb", [128, 1], F32, kind="ExternalInput")
    wv_d = nc.dram_tensor("wv", [128, KCH, 64], BF16, kind="ExternalInput")
    wvb_d = nc.dram_tensor("wvb", [1, 64], BF16, kind="ExternalInput")
    wo_d = nc.dram_tensor("wo", [128, 4, HID], BF16, kind="ExternalInput")
    cos_d = nc.dram_tensor("cos", [128, L], BF16, kind="ExternalInput")
    sin_d = nc.dram_tensor("sin", [128, L], BF16, kind="ExternalInput")
    mask_d = nc.dram_tensor("mask", [128, 2, 128], BF16, kind="ExternalInput")
    out_d = nc.dram_tensor("out", [L // 2, HID], BF16, kind="ExternalOutput")

    EXP = mybir.ActivationFunctionType.Exp
    LN = mybir.ActivationFunctionType.Ln
    IDT = mybir.ActivationFunctionType.Identity

    with tile.TileContext(nc) as tc:
        with (
            tc.tile_pool(name="const", bufs=1) as cpool,
            tc.tile_pool(name="qt", bufs=4) as qtpool,
            tc.tile_pool(name="per", bufs=1) as perpool,
            tc.tile_pool(name="ot", bufs=4) as otpool,
            tc.tile_pool(name="rraw", bufs=2) as rrawp,
            tc.tile_pool(name="rt1", bufs=2) as rt1p,
            tc.tile_pool(name="rt2", bufs=2) as rt2p,
            tc.tile_pool(name="ptp", bufs=8) as ptp,
            tc.tile_pool(name="nrm", bufs=4) as nrm,
            tc.tile_pool(name="osb", bufs=4) as osbp,
            tc.tile_pool(name="ps_sp", bufs=4, space="PSUM") as ps_sp,
            tc.tile_pool(name="ps_o", bufs=2, space="PSUM") as ps_o,
            tc.tile_pool(name="dram", bufs=1, space="DRAM") as drpool,
        ):
            # ---- constants / inputs to SBUF (first consumers first) ----
            wk = cpool.tile([128, KCH, 128], BF16, tag="wk")
            nc.sync.dma_start(wk[:], wk_d.ap())
            kb = cpool.tile([128, 1], F32, tag="kb")
            nc.sync.dma_start(kb[:], kb_d.ap())
            xt = cpool.tile([128, KCH, L], BF16, tag="xt")
            # band-major so K-proj m=0 can start after ~1/4 of the x DMA
            for m in range(4):
                for k in range(KCH):
                    ms = bass.ts(m, 512)
                    nc.sync.dma_start(xt[:, k, ms], xt_d.ap()[:, k, ms])
            cosb = cpool.tile([128, L], BF16, tag="cosb")
            nc.sync.dma_start(cosb[:], cos_d.ap())
            sinb = cpool.tile([128, L], BF16, tag="sinb")
            nc.sync.dma_start(sinb[:], sin_d.ap())
            wv = cpool.tile([128, KCH, 64], BF16, tag="wv")
            nc.sync.dma_start(wv[:], wv_d.ap())
            wvb = cpool.tile([1, 64], BF16, tag="wvb")
            nc.sync.dma_start(wvb[:], wvb_d.ap())
            wq = cpool.tile([128, KCH, 448], BF16, tag="wq")
            nc.sync.dma_start(wq[:], wq_d.ap())
            qb = cpool.tile([128, 4], F32, tag="qb")
            nc.sync.dma_start(qb[:], qb_d.ap())
            msk = cpool.tile([128, 2, 128], BF16, tag="msk")
            nc.sync.dma_start(msk[:], mask_d.ap())
            wo = cpool.tile([128, 4, HID], BF16, tag="wo")
            nc.sync.dma_start(wo[:], wo_d.ap())
            ones_row = cpool.tile([1, L], BF16, tag="ones_row")
            nc.vector.memset(ones_row[:], 1.0)
            ones64 = cpool.tile([1, 64], BF16, tag="ones64")
            nc.vector.memset(ones64[:], 1.0)

            # PE warmup: ~7.5us of dummy matmuls while the input DMAs land,
            # so the HAM clock-gate reaches 8/8 before the projections start.
            warm = ps_o.tile([128, 1024], F32, tag="o", name="warm")
            for _ in range(70):
                nc.tensor.matmul(warm[:, 0:128], ones_row[0:1, 0:128],
                                 ones_row[0:1, 0:128], start=True, stop=True)

            partials = [
                drpool.tile([n, HID], BF16, tag=f"partial{k}",
                            name=f"partial{k}")
                for k, (_, n) in enumerate(CH_ROWS)
            ]
            shards = [
                drpool.tile([n // 2, HID], BF16, tag=f"shard{k}",
                            name=f"shard{k}")
                for k, (_, n) in enumerate(CH_ROWS)
            ]

            def rope(dst, raw, P, hs):
                """dst[0:P, hs] = (raw*cos + rotate_half(raw)*sin)[:, hs] in
                bf16 (DVE 2x mode), called per half-L so qt bands are ready
                early; the rotate_half partition swap is cross-offset muls
                against the row-swapped sign-folded sin."""
                t1 = rt1p.tile([128, L // 2], BF16, tag="t1")
                nc.vector.tensor_mul(t1[0:P, :], raw[0:P, hs], cosb[0:P, hs])
                t2 = rt2p.tile([128, L // 2], BF16, tag="t2")
                for b in range(P // 32):
                    s = 32 * (b ^ 1)
                    nc.vector.tensor_mul(
                        t2[32 * b : 32 * b + 32, :],
                        raw[s : s + 32, hs],
                        sinb[s : s + 32, hs],
                    )
                nc.gpsimd.tensor_add(dst[0:P, hs], t1[0:P, :], t2[0:P, :])

            # ---- K^T projection (duplicated across partitions) + RoPE ----
            kt = perpool.tile([128, L], BF16, tag="kt")
            kraw = rrawp.tile([128, L], BF16, tag="raw", name="kraw")
            for m in range(4):
                ms = bass.ts(m, 512)
                kp = ps_sp.tile([128, 512], F32, tag="sp", name="kp")
                for k in range(KCH):
                    nc.tensor.matmul(kp[:, :], wk[:, k, :], xt[:, k, ms],
                                     start=(k == 0), stop=(k == KCH - 1))
                # PSUM->SBUF bf16 cast with the k bias folded in
                nc.scalar.activation(kraw[:, ms], kp[:, :], IDT,
                                     bias=kb[:, 0:1])
                if m % 2 == 1:
                    hs = bass.ts(m // 2, 1024)
                    # rope the single kv head (rows 0:64), then duplicate to
                    # rows 64:128 (for the h64 S row-group) with a cheap
                    # SBUF->SBUF DMA instead of doubled DVE mul work
                    rope(kt, kraw, 64, hs)
                    nc.sync.dma_start(kt[64:128, hs], kt[0:64, hs])

            # ---- Q^T projections + RoPE (head pairs on 128 partitions),
            # ---- pair 0 first so band-0 attention starts before V lands ----
            qts = []

            def qproj(p):
                P = 128 if p < 3 else 64
                ns = bass.ds(128 * p, P)
                qt = qtpool.tile([128, L], BF16, tag="qt", name=f"qt{p}")
                qts.append(qt)
                qraw = rrawp.tile([128, L], BF16, tag="raw", name=f"qraw{p}")
                for m in range(4):
                    ms = bass.ts(m, 512)
                    qp = ps_sp.tile([128, 512], F32, tag="sp", name="qp")
                    for k in range(KCH):
                        nc.tensor.matmul(qp[0:P, :], wq[:, k, ns],
                                         xt[:, k, ms],
                                         start=(k == 0), stop=(k == KCH - 1))
                    nc.scalar.activation(qraw[0:P, ms], qp[0:P, :], IDT,
                                         bias=qb[0:P, p : p + 1])
                    if m % 2 == 1:
                        rope(qt, qraw, P, bass.ts(m // 2, 1024))

            qproj(0)

            # ---- V projection (natural layout + ones column) ----
            vt = perpool.tile([128, NJC, 65], BF16, tag="vt")
            nc.vector.memset(vt[:, :, 64:65], 1.0)
            for mt in range(NJC):
                vp = ps_o.tile([128, 1024], F32, tag="o", name="vp")
                for k in range(KCH):
                    nc.tensor.matmul(vp[:, 0:64], xt[:, k, bass.ts(mt, 128)],
                                     wv[:, k, :], start=(k == 0), stop=False)
                nc.tensor.matmul(vp[:, 0:64], ones_row[0:1, bass.ts(mt, 128)],
                                 wvb[0:1, :], start=False, stop=True)
                nc.scalar.copy(vt[:, mt, 0:64], vp[:, 0:64])

            for p in range(1, 4):
                qproj(p)

            # ---- attention, head pairs packed on PE rows 0:64 / 64:128 ----
            # ib-major so each 512-row band of O^T completes early and its
            # o_proj + ReduceScatter chunk overlaps the next band's attention
            otp = [
                otpool.tile([128, L], BF16, tag="ot", name=f"otp{i}")
                for i in range(4)
            ]
            # Deferred emission: each pair's normalization and each band's
            # o_proj chunks are queued as closures and emitted one-per-jc
            # inside LATER pairs' attention streams, so the PE FIFO always
            # has S/PV work queued ahead of instructions that wait on the
            # ScalarE Ln/Exp chain (measured ~2-3us PE stall per pair-band
            # otherwise, plus ~10us ScalarE bubbles at band boundaries).
            deferred = []

            def flush_one():
                if deferred:
                    deferred.pop(0)()

            def norm_ln_closure(oab, has_b):
                # stage 1: just the Ln (its input is ready when flushed, so
                # it never blocks; the PE-side broadcast comes >=1 jc later)
                lnd = nrm.tile([1, 1024], BF16, tag="lnd")

                def emit():
                    W = 1024 if has_b else 512
                    nc.scalar.activation(lnd[0:1, 0:W], oab[64:65, 0:W], LN)
                return lnd, emit

            def norm_rb_closure(p, ib, oab, lnd, has_b):
                def emit():
                    rba = ps_sp.tile([128, 512], F32, tag="sp", name="rb")
                    nc.tensor.matmul(rba[0:64, :], ones64[0:1, :],
                                     lnd[0:1, 0:512], start=True, stop=True)
                    rbs = nrm.tile([64, 1024], BF16, tag="rbs")
                    nc.scalar.activation(rbs[:, 0:512], rba[0:64, :], EXP,
                                         scale=-1.0)
                    if has_b:
                        rbb = ps_sp.tile([128, 512], F32, tag="sp", name="rb")
                        nc.tensor.matmul(rbb[0:64, :], ones64[0:1, :],
                                         lnd[0:1, 512:1024],
                                         start=True, stop=True)
                        nc.scalar.activation(rbs[:, 512:1024], rbb[0:64, :],
                                             EXP, scale=-1.0)
                    ibb = bass.ts(ib, 512)
                    nc.vector.tensor_mul(otp[p][0:64, ibb],
                                         oab[0:64, 0:512], rbs[:, 0:512])
                    if has_b:
                        nc.vector.tensor_mul(otp[p][64:128, ibb],
                                             oab[0:64, 512:1024],
                                             rbs[:, 512:1024])
                return emit

            def _osb_dma(mt, ch, op_):
                kc = _kc_of(mt)
                osb = osbp.tile([128, 448], BF16, tag="osb")
                if ch == 0:
                    nc.scalar.copy(osb[:, :], op_[:, 0:448])
                else:
                    nc.vector.tensor_copy(osb[:, :], op_[:, 0:448])
                row0 = 128 * mt - CH_ROWS[kc][0]
                csl = bass.ts(ch, 448)
                nc.sync.dma_start(
                    partials[kc][bass.ds(row0, 128), csl], osb[:, :]
                )
                if ch == 1 and (
                    128 * mt + 128 == CH_ROWS[kc][0] + CH_ROWS[kc][1]
                ):
                    nc.gpsimd.collective_compute(
                        "ReduceScatter",
                        mybir.AluOpType.add,
                        ins=[partials[kc].opt()],
                        outs=[shards[kc].opt()],
                        replica_groups=[[0, 1], [2, 3], [4, 5], [6, 7]],
                    )
                    nc.sync.dma_start(
                        out_d.ap()[
                            bass.ds(CH_ROWS[kc][0] // 2, CH_ROWS[kc][1] // 2),
                            :,
                        ],
                        shards[kc][:, :],
                    )

            for ib in range(NIB):
                i0 = 512 * ib
                for p in range(4):
                    qt = qts[p]
                    has_b = p < 3
                    schr_b = has_b and p < NSCHR
                    # sides a/b accumulate in one 2-bank PSUM tile:
                    # cols 0:512 side a, 512:1024 side b; row 64 = denominators
                    oab = ps_o.tile([128, 1024], F32, tag="o", name="oab")
                    njc = 4 * ib + 4
                    for jc in range(njc):
                        t = jc - 4 * ib  # >=0 on the diagonal blocks
                        c0 = 128 * t if t >= 0 else 0
                        cw = 512 - c0
                        cs = bass.ds(c0, cw)
                        isl = bass.ds(i0 + c0, cw)
                        jsl = bass.ts(jc, 128)
                        sp_a = ps_sp.tile([128, 512], F32, tag="sp",
                                          name="sp_a")
                        nc.tensor.matmul(sp_a[:, cs], kt[0:64, jsl],
                                         qt[0:64, isl], start=True, stop=True)
                        if has_b:
                            sp_b = ps_sp.tile([128, 512], F32, tag="sp",
                                              name="sp_b")
                            nc.tensor.matmul(sp_b[:, cs],
                                             kt[64:128, jsl], qt[64:128, isl],
                                             start=True, stop=True)
                        pt = ptp.tile([128, 1024], BF16, tag="pt")
                        # --- exp: side a on ScalarE, side b on ScalarE or
                        # --- VectorE (Schraudolph bit-trick) ---
                        nc.scalar.activation(pt[:, 0:512][:, cs],
                                             sp_a[:, cs], EXP, scale=0.125)
                        if schr_b:
                            nc.vector.tensor_scalar(
                                pt[:, 512:1024][:, cs].bitcast(I16),
                                sp_b[:, cs],
                                SCHR_A, SCHR_B,
                                mybir.AluOpType.mult, mybir.AluOpType.add,
                            )
                        elif has_b:
                            nc.scalar.activation(pt[:, 512:1024][:, cs],
                                                 sp_b[:, cs], EXP,
                                                 scale=0.125)
                        if t >= 0:
                            # zero the invalid upper triangle of the diagonal
                            # sub-tile (0/1 mask)
                            pt3 = pt.rearrange("p (s c) -> p s c", s=2)
                            dcs = bass.ds(c0, 128)
                            if MASK_DVE:
                                if has_b:
                                    nc.vector.tensor_mul(pt3[:, :, dcs],
                                                         pt3[:, :, dcs],
                                                         msk[:, :, :])
                                else:
                                    nc.vector.tensor_mul(pt[:, dcs],
                                                         pt[:, dcs],
                                                         msk[:, 0, :])
                            else:
                                nc.gpsimd.tensor_mul(pt[:, dcs], pt[:, dcs],
                                                     msk[:, 0, :])
                                if has_b:
                                    dcs2 = bass.ds(512 + c0, 128)
                                    nc.gpsimd.tensor_mul(pt[:, dcs2],
                                                         pt[:, dcs2],
                                                         msk[:, 0, :])
                        nc.tensor.matmul(oab[0:65, 0:512][:, cs], vt[:, jc, :],
                                         pt[:, cs],
                                         start=(jc == 0), stop=(jc == njc - 1))
                        if has_b:
                            nc.tensor.matmul(oab[0:65, 512:1024][:, cs],
                                             vt[:, jc, :],
                                             pt[:, 512:1024][:, cs],
                                             start=(jc == 0), stop=(jc == njc - 1))
                        if jc >= 1:
                            flush_one()
                    if p < 3:
                        lnd, ln_emit = norm_ln_closure(oab, has_b)
                        deferred.append(ln_emit)
                        deferred.append(norm_rb_closure(p, ib, oab, lnd, has_b))
                    else:
                        oab3, lnd3 = oab, None
                while deferred:
                    flush_one()
                # --- band end: p3's Ln overlaps the first two o_proj groups'
                # p0-p2 matmuls (their otp inputs are ready); p3's broadcast +
                # muls then land before the groups' p3 matmuls need them.
                # o_proj itself stays inline so the RS chunks fire early. ---
                lnd3, ln3_emit = norm_ln_closure(oab3, False)
                ln3_emit()
                ops_open = []
                for mt in range(4 * ib, 4 * ib + 2):
                    msl = bass.ts(mt, 128)
                    op_ = ps_sp.tile([128, 512], F32, tag="sp", name="opj")
                    for p in range(3):
                        nc.tensor.matmul(op_[:, 0:448], otp[p][0:128, msl],
                                         wo[0:128, p, bass.ts(0, 448)],
                                         start=(p == 0), stop=False)
                    ops_open.append((mt, op_))
                norm_rb_closure(3, ib, oab3, lnd3, False)()
                for mt, op_ in ops_open:
                    msl = bass.ts(mt, 128)
                    nc.tensor.matmul(op_[:, 0:448], otp[3][0:64, msl],
                                     wo[0:64, 3, bass.ts(0, 448)],
                                     start=False, stop=True)
                    _osb_dma(mt, 0, op_)
                for mt in range(4 * ib, 4 * ib + 4):
                    for ch in range(2):
                        if ch == 0 and mt < 4 * ib + 2:
                            continue  # emitted above
                        msl = bass.ts(mt, 128)
                        op_ = ps_sp.tile([128, 512], F32, tag="sp",
                                         name="opj")
                        csl = bass.ts(ch, 448)
                        for p in range(4):
                            P = 128 if p < 3 else 64
                            nc.tensor.matmul(
                                op_[:, 0:448], otp[p][0:P, msl],
                                wo[0:P, p, csl],
                                start=(p == 0), stop=(p == 3),
                            )
                        _osb_dma(mt, ch, op_)


    _fix_drains(nc)
    return nc


def _kpack(wT):
    """[896, N] f32 -> [128, 7, N] bf16 contiguous (k-chunked)."""
    n = wT.shape[1]
    return np.ascontiguousarray(
        wT.reshape(KCH, 128, n).transpose(1, 0, 2).astype(BF16NP)
    )


def _wopack(wo_s):
    """wo shard [896, 448] -> [128, 4, 896] bf16: per head-pair p,
    partitions hold that pair's 128 rows of woT (= wo_s.T)."""
    woT = wo_s.T  # [448, 896]
    out = np.zeros((128, 4, HID), dtype=BF16NP)
    for p in range(4):
        rows = woT[128 * p : min(128 * p + 128, 448)]
        out[: rows.shape[0], p, :] = rows.astype(BF16NP)
    return out


_CACHE = {}


def kernel(**inputs):
    x = np.asarray(inputs["x"], dtype=np.float32)
    cos = np.asarray(inputs["cos"], dtype=np.float32)
    sin = np.asarray(inputs["sin"], dtype=np.float32)
    mask = np.asarray(inputs["mask"], dtype=np.float32)
    wq = np.asarray(inputs["wq"], dtype=np.float32)
    bq = np.asarray(inputs["bq"], dtype=np.float32)
    wk = np.asarray(inputs["wk"], dtype=np.float32)
    bk = np.asarray(inputs["bk"], dtype=np.float32)
    wv = np.asarray(inputs["wv"], dtype=np.float32)
    bv = np.asarray(inputs["bv"], dtype=np.float32)
    wo = np.asarray(inputs["wo"], dtype=np.float32)

    cosT = np.ascontiguousarray(
        np.tile(cos[0, 0].T, (2, 1)).astype(BF16NP))  # [128, L]
    sinT = sin[0, 0].T  # [64, L]
    # sign-folded sin for the in-place rotate_half: out[32b:32b+32] reads
    # q[32(b^1):...] times these rows; rows 0:32 carry the minus sign
    # rows pre-swapped in 32-blocks: the rope mul reads raw[s:s+32] and
    # sin rows s:s+32 and writes t2 rows 32b (s = 32*(b^1)); rows carrying
    # the minus sign are the (pre-swap) first half of each head
    sinm = np.ascontiguousarray(
        np.tile(np.concatenate([sinT[32:64], -sinT[0:32]], axis=0),
                (2, 1)).astype(BF16NP)
    )
    mask_diag = (mask[0, 0, :128, :128].T == 0.0).astype(BF16NP)
    mask2 = np.ascontiguousarray(
        np.stack([mask_diag, mask_diag], axis=1))  # [128, 2, 128]

    in_maps = []
    for core in range(NCORES):
        b, g = divmod(core, NKV)
        wq_s = wq[448 * g : 448 * (g + 1)]
        bq_s = bq[448 * g : 448 * (g + 1)]
        wk_s = wk[64 * g : 64 * (g + 1)]
        bk_s = bk[64 * g : 64 * (g + 1)]
        wv_s = wv[64 * g : 64 * (g + 1)]
        bv_s = bv[64 * g : 64 * (g + 1)]
        wo_s = wo[:, 448 * g : 448 * (g + 1)]  # [896, 448]
        wk_dup = np.concatenate([wk_s, wk_s], axis=0)  # [128, 896]
        bk_dup = np.concatenate([bk_s, bk_s], axis=0)
        qb2 = np.zeros((128, 4), dtype=np.float32)
        for p in range(4):
            rows = bq_s[128 * p : min(128 * p + 128, 448)]
            qb2[: rows.shape[0], p] = rows
        in_maps.append({
            "xt": _kpack(x[b].T),
            "wq": _kpack(wq_s.T),
            "qb": qb2,
            "wk": _kpack(wk_dup.T),
            "kb": np.ascontiguousarray(bk_dup[:, None]),
            "wv": _kpack(wv_s.T),
            "wvb": bv_s.astype(BF16NP)[None, :],
            "wo": _wopack(wo_s),
            "cos": cosT,
            "sin": sinm,
            "mask": mask2,
        })

    if "nc" not in _CACHE:
        _CACHE["nc"] = build()
    trace = bool(os.environ.get("KERNEL_TRACE"))
    res = run_bass_kernel_spmd(
        _CACHE["nc"], in_maps, core_ids=list(range(NCORES)), trace=trace
    )
    global LAST_EXEC_NS
    LAST_EXEC_NS = res.exec_time_ns
    out = np.empty((B, L, HID), dtype=np.float32)
    for b in range(B):
        lo = res.results[2 * b]["out"].astype(np.float32)
        hi = res.results[2 * b + 1]["out"].astype(np.float32)
        for start, n in CH_ROWS:
            h = n // 2
            s2 = start // 2
            out[b, start : start + h] = lo[s2 : s2 + h]
            out[b, start + h : start + n] = hi[s2 : s2 + h]
    return out


LAST_EXEC_NS = None
